# revision 15
# baseline (speedup 1.0000x reference)
"""Causal GQA self-attention (B=2, S=2048, HID=2048, 16 q heads / 4 kv heads,
DH=128, interleaved RoPE) as a Trainium2 Bass/Tile kernel on 8 NeuronCores.

Sharding: core c -> (batch b = c // 4, kv-group g = c % 4). Each core computes
its batch's attention for the 4 q heads served by kv head g, plus the partial
output projection against Wo[:, group cols]; the host sums the 4 partials per
batch.

Device dataflow (per core), everything in "transposed" [feature, seq] layout:
  xT   [hid, s]   (host-pretransposed, bf16)
  qT   = WqT.T @ xT          (per head, psum f32)  -> rope -> bf16
  kT   = WkT.T @ xT          -> rope -> bf16
  v    [s, dh]    = xT.T @ WvT   (natural layout, bf16)
  sT   [s_k, s_q] = kT_tile.T @ qT_chunk      (one MM per tile, causal skip)
  pT   = exp(sT * scale) (* diag mask)        (ACT, bf16; softmax max-free)
  oT  += v_tile.T @ pT   ;  den += ones.T @ pT     (psum f32 accum)
  OT   = oT * (1/den)    (bf16)
  y   += OT_tile.T @ WoT_chunk  over heads    (psum f32) -> DRAM f32
"""

import os
import sys
from contextlib import ExitStack

import numpy as np

sys.path.insert(0, "/opt/trn_rl_repo")

import ml_dtypes

import concourse.bass as bass
import concourse.mybir as mybir
import concourse.tile as tile
from concourse import bacc
from concourse.bass_utils import run_bass_kernel_spmd

BF16 = mybir.dt.bfloat16
F32 = mybir.dt.float32
NP_BF16 = ml_dtypes.bfloat16

# problem constants
B, S, HID = 2, 2048, 2048
H, HK, DH = 16, 4, 128
NH = H // HK  # local q heads per core (= REP)
BASE = 10000.0
SCALE = 1.0 / float(np.sqrt(DH))

N_CORES = 8
QC = 512            # q-chunk (psum free dim)
KT = 128            # k-tile (partition dim)

LAST_RUN = {}


def build_nc(S=S, HID=HID, NH=NH):
    """Build the per-core Bass program. All cores run the same program (SPMD)."""
    HT = HID // 128        # hid tiles
    SC = S // QC           # s chunks
    ST = S // 128          # s tiles
    TPC = QC // 128        # 128-tiles per chunk

    nc = bacc.Bacc()

    d_xt = nc.declare_dram_parameter("xt", [128, SC * HT * QC], BF16, isOutput=False)
    d_wq = nc.declare_dram_parameter("wq", [128, HT * 128 * NH], BF16, isOutput=False)
    d_wkv = nc.declare_dram_parameter("wkv", [128, HT * 256], BF16, isOutput=False)
    d_wo = nc.declare_dram_parameter("wo", [128, NH * HID], BF16, isOutput=False)
    d_cos = nc.declare_dram_parameter("cos", [128, S], BF16, isOutput=False)
    d_sin = nc.declare_dram_parameter("sin", [128, S], BF16, isOutput=False)
    d_mask = nc.declare_dram_parameter("mask", [128, TPC * QC], BF16, isOutput=False)
    d_rt = nc.declare_dram_parameter("rt", [128, 128], BF16, isOutput=False)
    d_ones = nc.declare_dram_parameter("ones", [128, 128], BF16, isOutput=False)
    d_y = nc.declare_dram_parameter("y", [S, HID], F32, isOutput=True)

    with tile.TileContext(nc) as tc, ExitStack() as ctx:
        const = ctx.enter_context(tc.tile_pool(name="const", bufs=1))
        xtp = ctx.enter_context(tc.tile_pool(name="xtp", bufs=2))
        work = ctx.enter_context(tc.tile_pool(name="work", bufs=1))
        ptp = ctx.enter_context(tc.tile_pool(name="ptp", bufs=8))
        ysp = ctx.enter_context(tc.tile_pool(name="ysp", bufs=2))
        psum = ctx.enter_context(tc.tile_pool(name="psum", bufs=1, space="PSUM"))

        # --- persistent constants (DMA order = need order: K-proj inputs
        # first, Wo (used only by the output projection) last) ---
        wkv = const.tile([128, HT * 256], BF16, tag="wkv")
        nc.sync.dma_start(wkv[:, :], d_wkv[:, :])
        xq0 = xtp.tile([128, HT * QC], BF16, tag="xq")
        NDMA = 4  # split the chunk load into 4 large DMAs (issue cost ~0.7us each)
        W = HT * QC // NDMA
        for i in range(NDMA):
            nc.sync.dma_start(
                xq0[:, i * W:(i + 1) * W], d_xt[:, i * W:(i + 1) * W]
            )
        wq = const.tile([128, HT * 128 * NH], BF16, tag="wq")
        for j in range(NH):  # head-major: head j usable as soon as its slab lands
            nc.scalar.dma_start(
                wq[:, j * HT * 128:(j + 1) * HT * 128],
                d_wq[:, j * HT * 128:(j + 1) * HT * 128],
            )
        cos = const.tile([128, S], BF16, tag="cos")
        nc.scalar.dma_start(cos[:, :], d_cos[:, :])
        sin = const.tile([128, S], BF16, tag="sin")
        nc.scalar.dma_start(sin[:, :], d_sin[:, :])
        rt = const.tile([128, 128], BF16, tag="rt")
        nc.scalar.dma_start(rt[:, :], d_rt[:, :])
        ones = const.tile([128, 128], BF16, tag="ones")
        nc.scalar.dma_start(ones[:, :], d_ones[:, :])
        msk = const.tile([128, TPC * QC], BF16, tag="msk")
        nc.scalar.dma_start(msk[:, :], d_mask[:, :])
        wo = const.tile([128, NH * HID], BF16, tag="wo")
        nc.scalar.dma_start(wo[:, :], d_wo[:, :])

        # persistent activations
        q_ro = const.tile([128, NH * S], BF16, tag="q_ro")
        k_ro = const.tile([128, S], BF16, tag="k_ro")
        v_nat = const.tile([128, S], BF16, tag="v_nat")
        ot = const.tile([128, NH * S], BF16, tag="ot")

        def rope(raw, out_slice, c):
            """raw: [128, QC] f32 sbuf tile (pre-rope head block, dh on
            partitions). Writes bf16 roped output to out_slice."""
            rq = psum.tile([128, QC], F32, tag="pj", bufs=2)
            nc.tensor.matmul(rq[:, :], rt[:, :], raw[:, :], start=True, stop=True)
            t1 = work.tile([128, QC], F32, tag="t1", bufs=2)
            nc.vector.tensor_tensor(
                t1[:, :], raw[:, :], cos[:, c * QC:(c + 1) * QC], mybir.AluOpType.mult
            )
            t2 = work.tile([128, QC], F32, tag="t2", bufs=2)
            nc.vector.tensor_tensor(
                t2[:, :], rq[:, :], sin[:, c * QC:(c + 1) * QC], mybir.AluOpType.mult
            )
            nc.vector.tensor_tensor(out_slice, t1[:, :], t2[:, :], mybir.AluOpType.add)

        for c in range(SC):
            # ---- stream in x^T for this s-chunk: xq[:, i*QC:(i+1)*QC] is
            # [hid-tile i (128 partitions), QC seq cols]
            if c == 0:
                xq = xq0
            else:
                xq = xtp.tile([128, HT * QC], BF16, tag="xq")
                W = HT * QC // 4
                for i in range(4):
                    nc.sync.dma_start(
                        xq[:, i * W:(i + 1) * W],
                        d_xt[:, c * HT * QC + i * W:c * HT * QC + (i + 1) * W],
                    )

            # ---- K projection + rope for chunk c
            ps = psum.tile([128, QC], F32, tag="pj", bufs=2)
            for i in range(HT):
                nc.tensor.matmul(
                    ps[:, :],
                    wkv[:, i * 256:i * 256 + 128],
                    xq[:, i * QC:(i + 1) * QC],
                    start=(i == 0),
                    stop=(i == HT - 1),
                )
            kraw = work.tile([128, QC], BF16, tag="raw", bufs=3)
            nc.vector.tensor_copy(kraw[:, :], ps[:, :])
            rope(kraw, k_ro[:, c * QC:(c + 1) * QC], c)

            # ---- Q projections + rope for chunk c (per local head)
            for j in range(NH):
                ps = psum.tile([128, QC], F32, tag="pj", bufs=2)
                for i in range(HT):
                    nc.tensor.matmul(
                        ps[:, :],
                        wq[:, (j * HT + i) * 128:(j * HT + i + 1) * 128],
                        xq[:, i * QC:(i + 1) * QC],
                        start=(i == 0),
                        stop=(i == HT - 1),
                    )
                qraw = work.tile([128, QC], BF16, tag="raw", bufs=3)
                nc.vector.tensor_copy(qraw[:, :], ps[:, :])
                rope(qraw, q_ro[:, j * S + c * QC:j * S + (c + 1) * QC], c)

            # ---- V projection for chunk c (natural [s, dh] layout)
            for t in range(TPC):
                st = c * TPC + t  # global s-tile
                ps = psum.tile([128, 128], F32, tag="pj", bufs=2)
                for i in range(HT):
                    nc.tensor.matmul(
                        ps[:, :],
                        xq[:, i * QC + t * 128:i * QC + (t + 1) * 128],
                        wkv[:, i * 256 + 128:i * 256 + 256],
                        start=(i == 0),
                        stop=(i == HT - 1),
                    )
                nc.vector.tensor_copy(v_nat[:, st * 128:(st + 1) * 128], ps[:, :])

            # ---- attention for q-chunk c, all local heads
            nk = (c + 1) * TPC  # causal: k tiles 0 .. nk-1
            for j in range(NH):
                oacc = psum.tile([128, QC], F32, tag="acc", bufs=3)
                sacc = psum.tile([128, QC], F32, tag="acc", bufs=3)
                for kt in range(nk):
                    sps = psum.tile([128, QC], F32, tag="sps", bufs=3)
                    nc.tensor.matmul(
                        sps[:, :],
                        k_ro[:, kt * 128:(kt + 1) * 128],
                        q_ro[:, j * S + c * QC:j * S + (c + 1) * QC],
                        start=True,
                        stop=True,
                    )
                    pt = ptp.tile([128, QC], BF16, tag="pt")
                    nc.scalar.activation(
                        pt[:, :], sps[:, :], mybir.ActivationFunctionType.Exp,
                        bias=0.0, scale=SCALE,
                    )
                    dj = kt - c * TPC
                    if dj >= 0:  # diagonal tile: apply causal mask
                        nc.vector.tensor_tensor(
                            pt[:, :], pt[:, :], msk[:, dj * QC:(dj + 1) * QC],
                            mybir.AluOpType.mult,
                        )
                    nc.tensor.matmul(
                        sacc[:, :],
                        ones[:, :],
                        pt[:, :],
                        start=(kt == 0),
                        stop=(kt == nk - 1),
                    )
                    nc.tensor.matmul(
                        oacc[:, :],
                        v_nat[:, kt * 128:(kt + 1) * 128],
                        pt[:, :],
                        start=(kt == 0),
                        stop=(kt == nk - 1),
                    )
                rec = work.tile([128, QC], F32, tag="rec", bufs=2)
                nc.vector.reciprocal_approx_fast(out=rec[:, :], in_=sacc[:, :])
                nc.vector.tensor_tensor(
                    ot[:, j * S + c * QC:j * S + (c + 1) * QC],
                    oacc[:, :], rec[:, :], mybir.AluOpType.mult,
                )

            # ---- output projection for the s-tiles of chunk c
            for t in range(TPC):
                st = c * TPC + t
                yst = ysp.tile([128, HID], F32, tag="yst")
                for ho in range(HID // QC):
                    yps = psum.tile([128, QC], F32, tag="sps", bufs=3)
                    for j in range(NH):
                        nc.tensor.matmul(
                            yps[:, :],
                            ot[:, j * S + st * 128:j * S + (st + 1) * 128],
                            wo[:, j * HID + ho * QC:j * HID + (ho + 1) * QC],
                            start=(j == 0),
                            stop=(j == NH - 1),
                        )
                    if ho % 2 == 0:
                        nc.scalar.mul(yst[:, ho * QC:(ho + 1) * QC], yps[:, :], 1.0)
                    else:
                        nc.vector.tensor_copy(yst[:, ho * QC:(ho + 1) * QC], yps[:, :])
                nc.sync.dma_start(
                    d_y[st * 128:(st + 1) * 128, :HID // 2], yst[:, :HID // 2]
                )
                nc.sync.dma_start(
                    d_y[st * 128:(st + 1) * 128, HID // 2:], yst[:, HID // 2:]
                )

    if not nc.is_finalized():
        nc.finalize()
    return nc


def host_prep_x(x_b, S=S, HID=HID):
    """x [S, HID] -> device xT layout [128, SC*HT*QC] (bf16)."""
    HT = HID // 128
    SC = S // QC
    xx = x_b.astype(NP_BF16)
    # xt[p, ((c*HT)+i)*QC + s] = x[c*QC+s, i*128+p]
    return np.ascontiguousarray(
        xx.reshape(SC, QC, HT, 128).transpose(3, 0, 2, 1).reshape(128, SC * HT * QC)
    )


def host_prep_tables(pos0, S=S):
    """RoPE cos/sin tables, rotation matrix, diag masks, ones (shared)."""
    TPC = QC // 128
    inv_freq = 1.0 / (BASE ** (np.arange(0, DH, 2, dtype=np.float32) / DH))
    freqs = pos0.astype(np.float32)[:, None] * inv_freq[None, :]  # [S, 64]
    emb = np.concatenate([freqs, freqs], axis=-1)  # [S, DH]
    cosT = np.ascontiguousarray(np.cos(emb).T.astype(NP_BF16))  # [128, S]
    sinT = np.ascontiguousarray(np.sin(emb).T.astype(NP_BF16))

    # R^T for interleaved rotate_half: rh = R @ q, R[2i,2i+1]=-1, R[2i+1,2i]=1
    R = np.zeros((DH, DH), dtype=np.float32)
    ii = np.arange(0, DH, 2)
    R[ii, ii + 1] = -1.0
    R[ii + 1, ii] = 1.0
    rT = np.ascontiguousarray(R.T.astype(NP_BF16))

    # diagonal causal masks in [k, q] layout: mask[kk, dj*QC + qq] = qq >= kk + 128*dj
    kk = np.arange(128)[:, None]
    qq = np.arange(QC)[None, :]
    mask = np.ascontiguousarray(np.concatenate(
        [(qq >= kk + 128 * dj) for dj in range(TPC)], axis=1
    ).astype(NP_BF16))
    ones = np.ones((128, 128), dtype=NP_BF16)
    return {"cos": cosT, "sin": sinT, "mask": mask, "rt": rT, "ones": ones}


def host_prep_weights(Wq, Wk, Wv, Wo, g, HID=HID, NH=NH):
    """Per-kv-group weight shards in device layouts (bf16)."""
    HT = HID // 128

    wq_s = Wq[NH * 128 * g:NH * 128 * (g + 1), :].astype(NP_BF16)  # [NH*128, HID]
    # wq[p, i*128*NH + j*128 + d2] -> for lhsT [hid, dh]: value Wq_s[j*128+d2, i*128+p]
    wq = np.ascontiguousarray(
        wq_s.reshape(NH, 128, HT, 128).transpose(3, 0, 2, 1).reshape(128, NH * HT * 128)
    )
    # wq[p, (j*HT + i)*128 + d] = wq_s[j, d, i, p] = Wq_s[j*128+d, i*128+p]

    wk_s = Wk[128 * g:128 * (g + 1), :].astype(NP_BF16)  # [128, HID]
    wv_s = Wv[128 * g:128 * (g + 1), :].astype(NP_BF16)
    kv = np.concatenate([wk_s, wv_s], axis=0)  # [256, HID]
    wkv = np.ascontiguousarray(
        kv.reshape(256, HT, 128).transpose(2, 1, 0).reshape(128, HT * 256)
    )
    # wkv[p, i*256 + u] = kv[u, i*128+p]  OK

    wo_s = Wo[:, NH * 128 * g:NH * 128 * (g + 1)].astype(NP_BF16)  # [HID, NH*128]
    wo = np.ascontiguousarray(
        wo_s.reshape(HID, NH, 128).transpose(2, 1, 0).reshape(128, NH * HID)
    )
    # wo[p, j*HID + o] = wo_s[o, j*128+p] = Wo[o, cols0 + j*128+p]  OK

    return {"wq": wq, "wkv": wkv, "wo": wo}


_NC_CACHE = {}


def kernel(x, position_ids, Wq, Wk, Wv, Wo):
    x = np.asarray(x, dtype=np.float32)
    position_ids = np.asarray(position_ids)
    Wq = np.asarray(Wq, dtype=np.float32)
    Wk = np.asarray(Wk, dtype=np.float32)
    Wv = np.asarray(Wv, dtype=np.float32)
    Wo = np.asarray(Wo, dtype=np.float32)
    assert x.shape == (B, S, HID), x.shape

    if "nc" not in _NC_CACHE:
        _NC_CACHE["nc"] = build_nc()
    nc = _NC_CACHE["nc"]

    pos0 = position_ids[0]  # reference uses row 0 for both batches
    tables = host_prep_tables(pos0)
    xts = [host_prep_x(x[b]) for b in range(B)]
    wshards = [host_prep_weights(Wq, Wk, Wv, Wo, g) for g in range(HK)]
    in_maps = []
    for c in range(N_CORES):
        b, g = divmod(c, HK)
        in_maps.append({"xt": xts[b], **wshards[g], **tables})

    trace = bool(int(os.environ.get("ATTN_TRACE", "0")))
    tmpdir = os.environ.get("ATTN_TRACE_DIR") or None
    if tmpdir is not None:
        LAST_RUN["n"] = LAST_RUN.get("n", 0) + 1
        tmpdir = os.path.join(tmpdir, f"run{LAST_RUN['n']}")
        os.makedirs(tmpdir, exist_ok=True)
    res = run_bass_kernel_spmd(
        nc, in_maps, list(range(N_CORES)), trace=trace, tmpdir=tmpdir
    )
    LAST_RUN["exec_time_ns"] = res.exec_time_ns
    LAST_RUN["mean_exec_time_ns"] = res.mean_exec_time_ns
    LAST_RUN["trace_dir"] = tmpdir

    out = np.zeros((B, S, HID), dtype=np.float32)
    for c in range(N_CORES):
        b = c // HK
        out[b] += res.results[c]["y"]
    return out
